# revision 2
# baseline (speedup 1.0000x reference)
"""Trainium2 Bass kernel v3 for dense transformer block nn_Block_68221260529679.

Layout: B=2, T=2048, D=2048, N=8 q-heads, K=1 kv-head, H=256, F=16384.

Sharding (8 NeuronCores): DP over batch (2 groups of 4) x T-split within the
group (4 chunks of 512 tokens).  Core c = 4*b + r handles batch b, tokens
[512r, 512r+512).  Every core computes the full k/v projection for its batch
(K=1 kv-head, cheap) and the full attention + MLP for its own 512 tokens with
the FULL weights.  Zero collectives; one SPMD program with no rank-dependent
control flow — all rank variation is carried by the input data (token chunk,
rope tables for the chunk, attention mask tiles).

Everything on device lives in transposed [feature, token] layout, so there are
no PE transposes at all:
  - x^T arrives from host (bf16); rmsnorm per-token stats are computed with
    ones-vector matmuls on the PE and broadcast back with a rank-1 matmul.
  - logits are computed transposed (partition = key, free = query); softmax
    normalization is deferred: exp(l - 8) without row-max, masked by a 0/1
    data mask, summed per query with a ones-matmul, and the reciprocal is
    folded into the attention-vector output columns.
  - o-proj, residuals, ffn-norm, gate/up/gelu and down-proj all stay in
    [feature, token] form; the host transposes the [D, 512] output back.

All matmuls bf16 with fp32 PSUM accumulation.  MLP runs in two F-halves so
the intermediate ff^T fits SBUF (64 KiB/partition per half).
"""

from contextlib import ExitStack

import numpy as np
import ml_dtypes

import concourse.bass as bass
import concourse.mybir as mybir
import concourse.tile as tile
from concourse import bacc
from concourse.masks import make_identity

F32 = mybir.dt.float32
BF16 = mybir.dt.bfloat16
AF = mybir.ActivationFunctionType
ALU = mybir.AluOpType

T, D, H, NH, F = 2048, 2048, 256, 8, 16384
CH = 512                 # tokens per core
DT = D // 128            # 16 d-blocks
TT = T // 128            # 16 t/s-tiles
HB = NH * H // 128       # 16 enc blocks
FB = F // 128            # 128 f-blocks
FH = FB // 2             # f-blocks per MLP pass
EXP_BIAS = -8.0          # folded stabilizer: exp(l - 8)

FULL_CFG = dict(version=3)


def build(cfg):
    REPS = cfg.get("reps", 1)
    nc = bacc.Bacc("TRN2", target_bir_lowering=False, debug=False,
                   num_devices=8)
    xt_ext = nc.dram_tensor("xt", [D, T], BF16, kind="ExternalInput").ap()
    xtq_ext = nc.dram_tensor("xtq", [D, CH], BF16, kind="ExternalInput").ap()
    wq_ext = nc.dram_tensor("wq", [HB, 128, DT, 128], BF16,
                            kind="ExternalInput").ap()
    wkv_ext = nc.dram_tensor("wkv", [128, DT, 2 * H], BF16,
                             kind="ExternalInput").ap()
    wo_ext = nc.dram_tensor("wo", [HB, 128, D], BF16,
                            kind="ExternalInput").ap()
    wg_ext = nc.dram_tensor("wg", [FB, 128, DT, 256], BF16,
                            kind="ExternalInput").ap()
    wl_ext = nc.dram_tensor("wl", [DT, 128, FB, 128], BF16,
                            kind="ExternalInput").ap()
    sin_ext = nc.dram_tensor("sin", [128, T], BF16, kind="ExternalInput").ap()
    cos_ext = nc.dram_tensor("cos", [128, T], BF16, kind="ExternalInput").ap()
    sinq_ext = nc.dram_tensor("sinq", [128, CH], BF16,
                              kind="ExternalInput").ap()
    cosq_ext = nc.dram_tensor("cosq", [128, CH], BF16,
                              kind="ExternalInput").ap()
    mm_ext = nc.dram_tensor("mmask", [128, TT, CH], BF16,
                            kind="ExternalInput").ap()
    out_ext = nc.dram_tensor("out", [D, CH], F32, kind="ExternalOutput").ap()
    DBG = cfg.get("debug", False)
    if DBG:
        dbg = {k: nc.dram_tensor(f"d_{k}", shp, dt, kind="ExternalOutput").ap()
               for k, shp, dt in [
                   ("rstd", [1, T], BF16), ("rbq", [128, CH], BF16),
                   ("rstdT", [128, TT], F32), ("kT", [128, 2, T], BF16),
                   ("v", [128, TT, H], BF16), ("qT", [128, HB, CH], BF16),
                   ("expl", [128, TT, CH], BF16),
                   ("encA", [128, HB, CH], BF16),
                   ("x2c", [128, DT, CH], BF16),
                   ("h2c", [128, DT, CH], BF16)]}

    with tile.TileContext(nc) as tc, ExitStack() as top:
        cons = top.enter_context(tc.tile_pool(name="cons", bufs=1))
        ones = cons.tile([128, 1], BF16)
        nc.vector.memset(ones, 1.0)
        ones_row = cons.tile([1, 128], BF16)
        nc.vector.memset(ones_row, 1.0)
        eps = cons.tile([1, 1], F32)
        nc.vector.memset(eps, 1e-6)
        ebias = cons.tile([128, 1], F32)
        nc.vector.memset(ebias, EXP_BIAS)
        ident = cons.tile([128, 128], BF16)
        make_identity(nc, ident)

        def colnorm_stats(src, ncols, p_sq, p_ps, out_bf):
            """rstd (bf16, [1, ncols]) for columns of src [128, DT, ncols]."""
            nchunk = ncols // 512
            ssqs = [p_ps.tile([1, 512], F32, tag=f"ssq{i}", bufs=1,
                              name=f"ssq{i}") for i in range(nchunk)]
            for kd in range(DT):
                sq = p_sq.tile([128, ncols], BF16, tag="sq")
                nc.scalar.activation(out=sq, in_=src[:, kd], func=AF.Square)
                for ci in range(nchunk):
                    nc.tensor.matmul(ssqs[ci], ones,
                                     sq[:, ci * 512:(ci + 1) * 512],
                                     start=kd == 0, stop=kd == DT - 1)
            for ci in range(nchunk):
                std = p_sq.tile([1, 512], F32, tag="std", name="std")
                nc.scalar.activation(out=std, in_=ssqs[ci], func=AF.Sqrt,
                                     bias=eps, scale=1.0 / D)
                with nc.allow_low_precision(reason="rstd in bf16 by design"):
                    nc.vector.reciprocal(
                        out=out_bf[:, ci * 512:(ci + 1) * 512], in_=std)

        for _rep in range(REPS):
            with ExitStack() as rep_sc:
                # long-lived pools, nested by live range (LIFO close order)
                p_x2 = rep_sc.enter_context(tc.tile_pool(name="p_x2",
                                                         bufs=1))
                x2c = p_x2.tile([128, DT, CH], BF16, tag="x2c")
                h2c = p_x2.tile([128, DT, CH], BF16, tag="h2c")

                with ExitStack() as res_sc:
                    p_res = res_sc.enter_context(
                        tc.tile_pool(name="p_res", bufs=1))
                    xres = p_res.tile([128, DT, CH], BF16, tag="xres")
                    encA = p_res.tile([128, HB, CH], BF16, tag="encA")

                    with ExitStack() as qkv_sc:
                        p_qkv = qkv_sc.enter_context(
                            tc.tile_pool(name="p_qkv", bufs=1))
                        qT = p_qkv.tile([128, HB, CH], BF16, tag="qT")
                        kT = p_qkv.tile([128, 2, T], BF16, tag="kT")
                        v_sb = p_qkv.tile([128, TT, H], BF16, tag="v")
                        rstdT = p_qkv.tile([128, TT], F32, tag="rstdT")

                        with ExitStack() as ab_sc:
                            p_ax = ab_sc.enter_context(
                                tc.tile_pool(name="p_ax", bufs=1))
                            xts = p_ax.tile([128, DT, T], BF16, tag="xts")
                            rb_sb = p_ax.tile([128, T], BF16, tag="rbsb")
                            rbq_sb = p_ax.tile([128, CH], BF16, tag="rbqsb")

                            # ---- Phase A: load x^T, column rmsnorm stats
                            with tc.tile_pool(name="pax", bufs=2) as pax, \
                                 tc.tile_pool(name="psax", bufs=1,
                                              space="PSUM") as psax:
                                nc.sync.dma_start(
                                    out=xts,
                                    in_=xt_ext.rearrange("(a p) t -> p a t",
                                                         p=128))
                                nc.sync.dma_start(
                                    out=xres,
                                    in_=xtq_ext.rearrange("(a p) t -> p a t",
                                                          p=128))
                                rstd = pax.tile([1, T], BF16, tag="rstd",
                                                bufs=1)
                                colnorm_stats(xts, T, pax, psax, rstd)
                                rstdq = pax.tile([1, CH], BF16, tag="rstdq",
                                                 bufs=1)
                                colnorm_stats(xres, CH, pax, psax, rstdq)
                                # broadcast rstd across partitions
                                for ci in range(4):
                                    pb = psax.tile([128, 512], F32,
                                                   tag="bcast", bufs=2,
                                                   name="bcast")
                                    csl = slice(ci * 512, (ci + 1) * 512)
                                    nc.tensor.matmul(pb, ones_row,
                                                     rstd[:, csl],
                                                     start=True, stop=True)
                                    nc.vector.tensor_copy(rb_sb[:, csl], pb)
                                pbq = psax.tile([128, CH], F32, tag="bcast",
                                                bufs=2, name="pbq")
                                nc.tensor.matmul(pbq, ones_row, rstdq,
                                                 start=True, stop=True)
                                nc.vector.tensor_copy(rbq_sb, pbq)
                                # rstdT[p, st] = rstd[128*st + p] via PE
                                # transposes of [1,128] row slices
                                for st in range(TT):
                                    pt = psax.tile([128, 1], BF16, tag="ptr",
                                                   bufs=2, name="ptr")
                                    nc.tensor.transpose(
                                        pt,
                                        rstd[:, st * 128:(st + 1) * 128],
                                        ident[:1, :1])
                                    nc.vector.tensor_copy(
                                        rstdT[:, st:st + 1], pt)
                                if DBG:
                                    nc.sync.dma_start(out=dbg["rstd"],
                                                      in_=rstd)
                                    nc.sync.dma_start(out=dbg["rbq"],
                                                      in_=rbq_sb)
                                    nc.sync.dma_start(out=dbg["rstdT"],
                                                      in_=rstdT)

                            # ---- Phase B: projections + rope + norm-fold
                            with ExitStack() as pb_sc:
                                pb_ = pb_sc.enter_context(
                                    tc.tile_pool(name="pb", bufs=2))
                                psb = pb_sc.enter_context(
                                    tc.tile_pool(name="psb", bufs=2,
                                                 space="PSUM"))
                                sin_sb = pb_.tile([128, T], BF16, tag="sin",
                                                  bufs=1)
                                nc.sync.dma_start(out=sin_sb, in_=sin_ext)
                                cos_sb = pb_.tile([128, T], BF16, tag="cos",
                                                  bufs=1)
                                nc.sync.dma_start(out=cos_sb, in_=cos_ext)
                                sinq_sb = pb_.tile([128, CH], BF16,
                                                   tag="sinq", bufs=1)
                                nc.sync.dma_start(out=sinq_sb, in_=sinq_ext)
                                cosq_sb = pb_.tile([128, CH], BF16,
                                                   tag="cosq", bufs=1)
                                nc.sync.dma_start(out=cosq_sb, in_=cosq_ext)
                                wkv_sb = pb_.tile([128, DT, 2 * H], BF16,
                                                  tag="wkv", bufs=1)
                                nc.sync.dma_start(out=wkv_sb, in_=wkv_ext)

                                def rope_scale(dst1, dst2, p1, p2, cs, sn,
                                               rb, ncols):
                                    """dst = rope(p1,p2) * rb (norm fold)."""
                                    t1 = pb_.tile([128, ncols], BF16,
                                                  tag="rp1")
                                    t2 = pb_.tile([128, ncols], BF16,
                                                  tag="rp2")
                                    nc.vector.tensor_tensor(t1, p1, cs,
                                                            op=ALU.mult)
                                    nc.vector.tensor_tensor(t2, p2, sn,
                                                            op=ALU.mult)
                                    nc.vector.tensor_tensor(t1, t1, t2,
                                                            op=ALU.subtract)
                                    nc.vector.tensor_tensor(dst1, t1, rb,
                                                            op=ALU.mult)
                                    nc.vector.tensor_tensor(t1, p2, cs,
                                                            op=ALU.mult)
                                    nc.vector.tensor_tensor(t2, p1, sn,
                                                            op=ALU.mult)
                                    nc.vector.tensor_tensor(t1, t1, t2,
                                                            op=ALU.add)
                                    nc.vector.tensor_tensor(dst2, t1, rb,
                                                            op=ALU.mult)

                                # k (full T) + rope + rstd fold
                                for ci in range(4):
                                    csl = slice(ci * 512, (ci + 1) * 512)
                                    p1 = psb.tile([128, 512], F32, tag="p1")
                                    p2 = psb.tile([128, 512], F32, tag="p2")
                                    for kd in range(DT):
                                        nc.tensor.matmul(
                                            p1, wkv_sb[:, kd, 0:128],
                                            xts[:, kd, csl],
                                            start=kd == 0, stop=kd == DT - 1)
                                    for kd in range(DT):
                                        nc.tensor.matmul(
                                            p2, wkv_sb[:, kd, 128:256],
                                            xts[:, kd, csl],
                                            start=kd == 0, stop=kd == DT - 1)
                                    rope_scale(kT[:, 0, csl], kT[:, 1, csl],
                                               p1, p2, cos_sb[:, csl],
                                               sin_sb[:, csl],
                                               rb_sb[:, csl], 512)
                                # v natural [s, h], rstd fold per partition
                                for st in range(TT):
                                    pv = psb.tile([128, H], F32, tag="pv")
                                    for kd in range(DT):
                                        nc.tensor.matmul(
                                            pv,
                                            xts[:, kd,
                                                st * 128:(st + 1) * 128],
                                            wkv_sb[:, kd, H:2 * H],
                                            start=kd == 0, stop=kd == DT - 1)
                                    nc.vector.tensor_scalar_mul(
                                        v_sb[:, st], pv, rstdT[:, st:st + 1])
                                # q (chunk) + rope + rstdq fold
                                for n in range(NH):
                                    p1 = psb.tile([128, CH], F32, tag="p1")
                                    p2 = psb.tile([128, CH], F32, tag="p2")
                                    for j, ph in ((0, p1), (1, p2)):
                                        hb = 2 * n + j
                                        wqt = pb_.tile([128, DT, 128], BF16,
                                                       tag="wqt", bufs=2)
                                        nc.sync.dma_start(out=wqt,
                                                          in_=wq_ext[hb])
                                        for kd in range(DT):
                                            nc.tensor.matmul(
                                                ph, wqt[:, kd], xres[:, kd],
                                                start=kd == 0,
                                                stop=kd == DT - 1)
                                    rope_scale(qT[:, 2 * n], qT[:, 2 * n + 1],
                                               p1, p2, cosq_sb, sinq_sb,
                                               rbq_sb, CH)
                                if DBG:
                                    nc.sync.dma_start(out=dbg["kT"], in_=kT)
                                    nc.sync.dma_start(out=dbg["v"], in_=v_sb)
                                    nc.sync.dma_start(out=dbg["qT"], in_=qT)

                        # ---- Phase C: attention (transposed logits) ----
                        with ExitStack() as pc_sc:
                            pc = pc_sc.enter_context(
                                tc.tile_pool(name="pc", bufs=2))
                            psc = pc_sc.enter_context(
                                tc.tile_pool(name="psc", bufs=2,
                                             space="PSUM"))
                            mm_sb = pc.tile([128, TT, CH], BF16, tag="mm",
                                            bufs=1)
                            nc.sync.dma_start(out=mm_sb, in_=mm_ext)
                            for n in range(NH):
                                explT = pc.tile([128, TT, CH], BF16,
                                                tag="explT")
                                rsum = psc.tile([1, CH], F32, tag="rsum")
                                for st in range(TT):
                                    ssl = slice(st * 128, (st + 1) * 128)
                                    pl = psc.tile([128, CH], F32, tag="pl")
                                    nc.tensor.matmul(pl, kT[:, 0, ssl],
                                                     qT[:, 2 * n],
                                                     start=True, stop=False)
                                    nc.tensor.matmul(pl, kT[:, 1, ssl],
                                                     qT[:, 2 * n + 1],
                                                     start=False, stop=True)
                                    nc.scalar.activation(out=explT[:, st],
                                                         in_=pl, func=AF.Exp,
                                                         bias=ebias)
                                    nc.vector.tensor_tensor(explT[:, st],
                                                            explT[:, st],
                                                            mm_sb[:, st],
                                                            op=ALU.mult)
                                    nc.tensor.matmul(rsum, ones,
                                                     explT[:, st],
                                                     start=st == 0,
                                                     stop=st == TT - 1)
                                rrec = pc.tile([1, CH], BF16, tag="rrec")
                                with nc.allow_low_precision(
                                        reason="softmax 1/sum bf16"):
                                    nc.vector.reciprocal(out=rrec, in_=rsum)
                                rb = psc.tile([128, CH], F32, tag="rb")
                                nc.tensor.matmul(rb, ones_row, rrec,
                                                 start=True, stop=True)
                                rb_c = pc.tile([128, CH], BF16, tag="rbc")
                                nc.vector.tensor_copy(rb_c, rb)
                                for m in range(2):
                                    pe_ = psc.tile([128, CH], F32, tag="enc")
                                    for st in range(TT):
                                        nc.tensor.matmul(
                                            pe_,
                                            v_sb[:, st,
                                                 m * 128:(m + 1) * 128],
                                            explT[:, st],
                                            start=st == 0, stop=st == TT - 1)
                                    nc.vector.tensor_tensor(
                                        encA[:, 2 * n + m], pe_, rb_c,
                                        op=ALU.mult)
                                if DBG and n == 0:
                                    nc.sync.dma_start(out=dbg["expl"],
                                                      in_=explT)
                            if DBG:
                                nc.sync.dma_start(out=dbg["encA"], in_=encA)

                    # ---- Phase D: o-proj + residual + ffn-norm ----
                    with ExitStack() as pd_sc:
                        pd = pd_sc.enter_context(
                            tc.tile_pool(name="pd", bufs=2))
                        with tc.tile_pool(name="psdo", bufs=1,
                                          space="PSUM") as psdo:
                            for half in range(2):
                                aps = [psdo.tile([128, CH], F32,
                                                 tag=f"ao{i}", bufs=1,
                                                 name=f"ao{i}")
                                       for i in range(8)]
                                for hb in range(HB):
                                    wot = pd.tile([128, 1024], BF16,
                                                  tag="wot", bufs=3)
                                    nc.sync.dma_start(
                                        out=wot,
                                        in_=wo_ext[hb][:, half * 1024:
                                                       (half + 1) * 1024])
                                    for i in range(8):
                                        nc.tensor.matmul(
                                            aps[i],
                                            wot[:, i * 128:(i + 1) * 128],
                                            encA[:, hb],
                                            start=hb == 0, stop=hb == HB - 1)
                                for i in range(8):
                                    kd = half * 8 + i
                                    nc.vector.tensor_tensor(
                                        x2c[:, kd], aps[i], xres[:, kd],
                                        op=ALU.add)
                        # ffn norm on x2c columns
                        with tc.tile_pool(name="psd2", bufs=1,
                                          space="PSUM") as psd2:
                            ssq2 = psd2.tile([1, CH], F32, tag="ssq2",
                                             bufs=1)
                            for kd in range(DT):
                                sq2 = pd.tile([128, CH], BF16, tag="sq2")
                                nc.scalar.activation(out=sq2, in_=x2c[:, kd],
                                                     func=AF.Square)
                                nc.tensor.matmul(ssq2, ones, sq2,
                                                 start=kd == 0,
                                                 stop=kd == DT - 1)
                            std2 = pd.tile([1, CH], F32, tag="std2", bufs=1)
                            nc.scalar.activation(out=std2, in_=ssq2,
                                                 func=AF.Sqrt, bias=eps,
                                                 scale=1.0 / D)
                            rstd2 = pd.tile([1, CH], BF16, tag="rstd2",
                                            bufs=1)
                            with nc.allow_low_precision(
                                    reason="rstd in bf16 by design"):
                                nc.vector.reciprocal(out=rstd2, in_=std2)
                            rb2 = psd2.tile([128, CH], F32, tag="rb2",
                                            bufs=1)
                            nc.tensor.matmul(rb2, ones_row, rstd2,
                                             start=True, stop=True)
                            rb2_sb = pd.tile([128, CH], BF16, tag="rb2sb",
                                             bufs=1)
                            nc.vector.tensor_copy(rb2_sb, rb2)
                            for kd in range(DT):
                                nc.vector.tensor_tensor(h2c[:, kd],
                                                        x2c[:, kd], rb2_sb,
                                                        op=ALU.mult)
                        if DBG:
                            nc.sync.dma_start(out=dbg["x2c"], in_=x2c)
                            nc.sync.dma_start(out=dbg["h2c"], in_=h2c)

                # ---- Phase E: MLP (two F-halves), output ----
                with ExitStack() as pe_sc:
                    pe = pe_sc.enter_context(tc.tile_pool(name="pe", bufs=2))
                    pse = pe_sc.enter_context(
                        tc.tile_pool(name="pse", bufs=2, space="PSUM"))
                    downA = pe.tile([128, DT, CH], BF16, tag="downA", bufs=1)
                    for half in range(2):
                        ffT = pe.tile([128, FH, CH], BF16, tag="ffT", bufs=1)
                        for fi in range(FH):
                            fb = half * FH + fi
                            wgf = pe.tile([128, DT, 256], BF16, tag="wgf",
                                          bufs=3)
                            nc.sync.dma_start(out=wgf, in_=wg_ext[fb])
                            gps = pse.tile([128, CH], F32, tag="gps")
                            ups = pse.tile([128, CH], F32, tag="ups")
                            for kd in range(DT):
                                nc.tensor.matmul(gps, wgf[:, kd, 0:128],
                                                 h2c[:, kd],
                                                 start=kd == 0,
                                                 stop=kd == DT - 1)
                            for kd in range(DT):
                                nc.tensor.matmul(ups, wgf[:, kd, 128:256],
                                                 h2c[:, kd],
                                                 start=kd == 0,
                                                 stop=kd == DT - 1)
                            ga = pe.tile([128, CH], BF16, tag="ga")
                            nc.scalar.activation(out=ga, in_=gps,
                                                 func=AF.Gelu_apprx_tanh)
                            nc.vector.tensor_tensor(ffT[:, fi], ga, ups,
                                                    op=ALU.mult)
                        for kd in range(DT):
                            wlt = pe.tile([128, FH, 128], BF16, tag="wlt",
                                          bufs=2)
                            nc.sync.dma_start(
                                out=wlt,
                                in_=wl_ext[kd][:, half * FH:(half + 1) * FH])
                            dps = pse.tile([128, CH], F32, tag="dps")
                            for fi in range(FH):
                                nc.tensor.matmul(dps, wlt[:, fi], ffT[:, fi],
                                                 start=fi == 0,
                                                 stop=fi == FH - 1)
                            if half == 0:
                                nc.vector.tensor_copy(downA[:, kd], dps)
                            else:
                                ot = pe.tile([128, CH], F32, tag="ot",
                                             bufs=3)
                                nc.vector.tensor_tensor(ot, dps,
                                                        downA[:, kd],
                                                        op=ALU.add)
                                nc.vector.tensor_tensor(ot, ot, x2c[:, kd],
                                                        op=ALU.add)
                                nc.sync.dma_start(
                                    out=out_ext[kd * 128:(kd + 1) * 128],
                                    in_=ot)
    nc.compile()
    return nc


# ---------------------------------------------------------------------------
# host side
# ---------------------------------------------------------------------------

def make_in_maps(cfg, x, positions, attn_mask, scale_attn, w_q, w_kv, w_o,
                 scale_ffn, w_gating, w_linear):
    bf = ml_dtypes.bfloat16
    B = np.asarray(x).shape[0]
    s1a = (1.0 + np.asarray(scale_attn, np.float32))[:, None]
    s1f = (1.0 + np.asarray(scale_ffn, np.float32))[:, None]

    # weights (shared by every core)
    wq_f = (np.asarray(w_q, np.float32) * s1a[None] * H ** -0.5)  # [N, D, H]
    Wq2 = np.concatenate(list(wq_f), axis=1)                      # [D, N*H]
    wq_t = np.ascontiguousarray(
        Wq2.reshape(DT, 128, HB, 128).transpose(2, 1, 0, 3).astype(bf))
    k_w = np.asarray(w_kv[0, 0], np.float32) * s1a
    v_w = np.asarray(w_kv[1, 0], np.float32) * s1a
    wkv_t = np.ascontiguousarray(
        np.concatenate([k_w, v_w], axis=1).astype(bf)
        .reshape(DT, 128, 2 * H).transpose(1, 0, 2))
    Wo2 = np.asarray(w_o, np.float32).reshape(NH * H, D)
    wo_t = np.ascontiguousarray(Wo2.reshape(HB, 128, D).astype(bf))
    gate = (np.asarray(w_gating[0], np.float32) * s1f).astype(bf)
    up = (np.asarray(w_gating[1], np.float32) * s1f).astype(bf)
    gate = gate.reshape(DT, 128, FB, 128).transpose(2, 1, 0, 3)
    up = up.reshape(DT, 128, FB, 128).transpose(2, 1, 0, 3)
    wg_t = np.ascontiguousarray(np.concatenate([gate, up], axis=3))
    wl_t = np.ascontiguousarray(
        np.asarray(w_linear, np.float32).astype(bf)
        .reshape(FB, 128, DT, 128).transpose(2, 1, 0, 3))

    freq = 10000.0 ** (2.0 / H * np.arange(H // 2, dtype=np.float32))
    mask = np.asarray(attn_mask)  # [B, 1, T, T] bool
    in_maps = []
    for c in range(8):
        b, r = divmod(c, 4)
        b = min(b, B - 1)
        xT = np.ascontiguousarray(
            np.asarray(x[b], np.float32).T.astype(bf))          # [D, T]
        pos = np.asarray(positions[b], np.float32)
        rad = pos[None, :] / freq[:, None]                       # [H/2, T]
        csl = slice(r * CH, (r + 1) * CH)
        # mmask[p, st, t] = mask[b, 0, chunk_t, s=128*st+p]
        mchunk = mask[b, 0, csl, :]                              # [CH, S]
        mm = np.ascontiguousarray(
            mchunk.T.reshape(TT, 128, CH).transpose(1, 0, 2)
            .astype(bf))
        in_maps.append({
            "xt": xT,
            "xtq": np.ascontiguousarray(xT[:, csl]),
            "wq": wq_t, "wkv": wkv_t, "wo": wo_t, "wg": wg_t, "wl": wl_t,
            "sin": np.ascontiguousarray(np.sin(rad).astype(bf)),
            "cos": np.ascontiguousarray(np.cos(rad).astype(bf)),
            "sinq": np.ascontiguousarray(np.sin(rad[:, csl]).astype(bf)),
            "cosq": np.ascontiguousarray(np.cos(rad[:, csl]).astype(bf)),
            "mmask": mm,
        })
    return in_maps


def assemble(cfg, results, B):
    out = np.empty((B, T, D), np.float32)
    for c in range(8):
        b, r = divmod(c, 4)
        if b >= B:
            continue
        out[b, r * CH:(r + 1) * CH, :] = results[c]["out"].T
    return out


# cached compiled program + jitted runner -----------------------------------

_CACHE = {}


def _get_runner(cfg_key, cfg):
    if cfg_key in _CACHE:
        return _CACHE[cfg_key]
    runner = _runner_from_nc(build(cfg))
    _CACHE[cfg_key] = runner
    return runner


def _runner_from_nc(nc):
    import jax
    from jax.experimental.shard_map import shard_map
    from jax.sharding import Mesh, PartitionSpec
    from concourse import bass2jax

    bass2jax.install_neuronx_cc_hook()

    partition_name = (nc.partition_id_tensor.name
                      if nc.partition_id_tensor else None)
    in_names, out_names, out_avals, zero_shapes = [], [], [], []
    for alloc in nc.m.functions[0].allocations:
        if not isinstance(alloc, mybir.MemoryLocationSet):
            continue
        name = alloc.memorylocations[0].name
        if alloc.kind == "ExternalInput":
            if name != partition_name:
                in_names.append(name)
        elif alloc.kind == "ExternalOutput":
            out_names.append(name)
            shape = tuple(alloc.tensor_shape)
            dtype = mybir.dt.np(alloc.dtype)
            out_avals.append(jax.core.ShapedArray(shape, dtype))
            zero_shapes.append((shape, dtype))
    n_params = len(in_names)
    all_in_names = in_names + out_names
    if partition_name is not None:
        all_in_names = all_in_names + [partition_name]

    def _body(*args):
        operands = list(args)
        if partition_name is not None:
            operands.append(bass2jax.partition_id_tensor())
        outs = bass2jax._bass_exec_p.bind(
            *operands,
            out_avals=tuple(out_avals),
            in_names=tuple(all_in_names),
            out_names=tuple(out_names),
            lowering_input_output_aliases=(),
            sim_require_finite=True,
            sim_require_nnan=True,
            nc=nc,
        )
        return tuple(outs)

    n_outs = len(out_names)
    donate = tuple(range(n_params, n_params + n_outs))
    devices = jax.devices()[:8]
    mesh = Mesh(np.asarray(devices), ("core",))
    in_specs = (PartitionSpec("core"),) * (n_params + n_outs)
    out_specs = (PartitionSpec("core"),) * n_outs
    sharded = jax.jit(
        shard_map(_body, mesh=mesh, in_specs=in_specs, out_specs=out_specs,
                  check_rep=False),
        donate_argnums=donate, keep_unused=True)

    class Runner:
        pass

    runner = Runner()
    runner.sharded = sharded
    runner.mesh = mesh
    runner.in_names = in_names
    runner.out_names = out_names
    runner.out_avals = out_avals
    runner.zero_shapes = zero_shapes

    def concat_inputs(in_maps):
        return [np.concatenate([np.asarray(m[name]) for m in in_maps],
                               axis=0) for name in in_names]

    def make_zeros():
        return [np.zeros((8 * s[0], *s[1:]), d) for s, d in zero_shapes]

    def split_outputs(out_arrs):
        return [
            {name: np.asarray(out_arrs[i]).reshape(8, *out_avals[i].shape)[c]
             for i, name in enumerate(out_names)}
            for c in range(8)
        ]

    runner.concat_inputs = concat_inputs
    runner.make_zeros = make_zeros
    runner.split_outputs = split_outputs

    def run(in_maps):
        out_arrs = sharded(*concat_inputs(in_maps), *make_zeros())
        return split_outputs(out_arrs)

    runner.run = run
    return runner


def run_cfg(cfg, inputs):
    cfg_key = tuple(sorted(cfg.items()))
    runner = _get_runner(cfg_key, cfg)
    in_maps = make_in_maps(cfg, **inputs)
    results = runner.run(in_maps)
    return assemble(cfg, results, np.asarray(inputs["x"]).shape[0])


def kernel(**inputs):
    return run_cfg(FULL_CFG, inputs)


# revision 3
# speedup vs baseline: 1.0227x; 1.0227x over previous
"""Trainium2 Bass kernel v3 for dense transformer block nn_Block_68221260529679.

Layout: B=2, T=2048, D=2048, N=8 q-heads, K=1 kv-head, H=256, F=16384.

Sharding (8 NeuronCores): DP over batch (2 groups of 4) x T-split within the
group (4 chunks of 512 tokens).  Core c = 4*b + r handles batch b, tokens
[512r, 512r+512).  Every core computes the full k/v projection for its batch
(K=1 kv-head, cheap) and the full attention + MLP for its own 512 tokens with
the FULL weights.  Zero collectives; one SPMD program with no rank-dependent
control flow — all rank variation is carried by the input data (token chunk,
rope tables for the chunk, attention mask tiles).

Everything on device lives in transposed [feature, token] layout, so there are
no PE transposes at all:
  - x^T arrives from host (bf16); rmsnorm per-token stats are computed with
    ones-vector matmuls on the PE and broadcast back with a rank-1 matmul.
  - logits are computed transposed (partition = key, free = query); softmax
    normalization is deferred: exp(l - 8) without row-max, masked by a 0/1
    data mask, summed per query with a ones-matmul, and the reciprocal is
    folded into the attention-vector output columns.
  - o-proj, residuals, ffn-norm, gate/up/gelu and down-proj all stay in
    [feature, token] form; the host transposes the [D, 512] output back.

All matmuls bf16 with fp32 PSUM accumulation.  MLP runs in two F-halves so
the intermediate ff^T fits SBUF (64 KiB/partition per half).
"""

from contextlib import ExitStack

import numpy as np
import ml_dtypes

import concourse.bass as bass
import concourse.mybir as mybir
import concourse.tile as tile
from concourse import bacc
from concourse.masks import make_identity

F32 = mybir.dt.float32
BF16 = mybir.dt.bfloat16
AF = mybir.ActivationFunctionType
ALU = mybir.AluOpType

T, D, H, NH, F = 2048, 2048, 256, 8, 16384
CH = 512                 # tokens per core
DT = D // 128            # 16 d-blocks
TT = T // 128            # 16 t/s-tiles
HB = NH * H // 128       # 16 enc blocks
FB = F // 128            # 128 f-blocks
FH = FB // 2             # f-blocks per MLP pass
EXP_BIAS = -8.0          # folded stabilizer: exp(l - 8)

FULL_CFG = dict(version=3)


def build(cfg):
    REPS = cfg.get("reps", 1)
    nc = bacc.Bacc("TRN2", target_bir_lowering=False, debug=False,
                   num_devices=8)
    xt_ext = nc.dram_tensor("xt", [D, T], BF16, kind="ExternalInput").ap()
    xtq_ext = nc.dram_tensor("xtq", [D, CH], BF16, kind="ExternalInput").ap()
    wq_ext = nc.dram_tensor("wq", [HB, 128, DT, 128], BF16,
                            kind="ExternalInput").ap()
    wkv_ext = nc.dram_tensor("wkv", [128, DT, 2 * H], BF16,
                             kind="ExternalInput").ap()
    wo_ext = nc.dram_tensor("wo", [HB, 128, D], BF16,
                            kind="ExternalInput").ap()
    wg_ext = nc.dram_tensor("wg", [FB, 128, DT, 256], BF16,
                            kind="ExternalInput").ap()
    wl_ext = nc.dram_tensor("wl", [DT, 128, FB, 128], BF16,
                            kind="ExternalInput").ap()
    sin_ext = nc.dram_tensor("sin", [128, T], BF16, kind="ExternalInput").ap()
    cos_ext = nc.dram_tensor("cos", [128, T], BF16, kind="ExternalInput").ap()
    sinq_ext = nc.dram_tensor("sinq", [128, CH], BF16,
                              kind="ExternalInput").ap()
    cosq_ext = nc.dram_tensor("cosq", [128, CH], BF16,
                              kind="ExternalInput").ap()
    mm_ext = nc.dram_tensor("mmask", [128, TT, CH], BF16,
                            kind="ExternalInput").ap()
    out_ext = nc.dram_tensor("out", [D, CH], F32, kind="ExternalOutput").ap()
    DBG = cfg.get("debug", False)
    if DBG:
        dbg = {k: nc.dram_tensor(f"d_{k}", shp, dt, kind="ExternalOutput").ap()
               for k, shp, dt in [
                   ("rstd", [1, T], BF16), ("rbq", [128, CH], BF16),
                   ("rstdT", [128, TT], F32), ("kT", [128, 2, T], BF16),
                   ("v", [128, TT, H], BF16), ("qT", [128, HB, CH], BF16),
                   ("expl", [128, TT, CH], BF16),
                   ("encA", [128, HB, CH], BF16),
                   ("x2c", [128, DT, CH], BF16),
                   ("h2c", [128, DT, CH], BF16)]}

    with tile.TileContext(nc) as tc, ExitStack() as top:
        cons = top.enter_context(tc.tile_pool(name="cons", bufs=1))
        ones = cons.tile([128, 1], BF16)
        nc.vector.memset(ones, 1.0)
        ones_row = cons.tile([1, 128], BF16)
        nc.vector.memset(ones_row, 1.0)
        eps = cons.tile([1, 1], F32)
        nc.vector.memset(eps, 1e-6)
        ebias = cons.tile([128, 1], F32)
        nc.vector.memset(ebias, EXP_BIAS)
        ident = cons.tile([128, 128], BF16)
        make_identity(nc, ident)

        def colnorm_stats(src, ncols, p_sq, p_ps, out_bf):
            """rstd (bf16, [1, ncols]) for columns of src [128, DT, ncols]."""
            nchunk = ncols // 512
            ssqs = [p_ps.tile([1, 512], F32, tag=f"ssq{i}", bufs=1,
                              name=f"ssq{i}") for i in range(nchunk)]
            for kd in range(DT):
                sq = p_sq.tile([128, ncols], BF16, tag="sq")
                nc.scalar.activation(out=sq, in_=src[:, kd], func=AF.Square)
                for ci in range(nchunk):
                    nc.tensor.matmul(ssqs[ci], ones,
                                     sq[:, ci * 512:(ci + 1) * 512],
                                     start=kd == 0, stop=kd == DT - 1)
            for ci in range(nchunk):
                std = p_sq.tile([1, 512], F32, tag="std", name="std")
                nc.scalar.activation(out=std, in_=ssqs[ci], func=AF.Sqrt,
                                     bias=eps, scale=1.0 / D)
                with nc.allow_low_precision(reason="rstd in bf16 by design"):
                    nc.vector.reciprocal(
                        out=out_bf[:, ci * 512:(ci + 1) * 512], in_=std)

        for _rep in range(REPS):
            with ExitStack() as rep_sc:
                # long-lived pools, nested by live range (LIFO close order)
                p_x2 = rep_sc.enter_context(tc.tile_pool(name="p_x2",
                                                         bufs=1))
                x2c = p_x2.tile([128, DT, CH], BF16, tag="x2c")
                h2c = p_x2.tile([128, DT, CH], BF16, tag="h2c")

                with ExitStack() as res_sc:
                    p_res = res_sc.enter_context(
                        tc.tile_pool(name="p_res", bufs=1))
                    xres = p_res.tile([128, DT, CH], BF16, tag="xres")
                    encA = p_res.tile([128, HB, CH], BF16, tag="encA")

                    with ExitStack() as qkv_sc:
                        p_qkv = qkv_sc.enter_context(
                            tc.tile_pool(name="p_qkv", bufs=1))
                        qT = p_qkv.tile([128, HB, CH], BF16, tag="qT")
                        kT = p_qkv.tile([128, 2, T], BF16, tag="kT")
                        v_sb = p_qkv.tile([128, TT, H], BF16, tag="v")
                        rstdT = p_qkv.tile([128, TT], F32, tag="rstdT")

                        with ExitStack() as ab_sc:
                            p_ax = ab_sc.enter_context(
                                tc.tile_pool(name="p_ax", bufs=1))
                            xts = p_ax.tile([128, DT, T], BF16, tag="xts")
                            rb_sb = p_ax.tile([128, T], BF16, tag="rbsb")
                            rbq_sb = p_ax.tile([128, CH], BF16, tag="rbqsb")

                            # ---- Phase A: load x^T, column rmsnorm stats
                            with tc.tile_pool(name="pax", bufs=2) as pax, \
                                 tc.tile_pool(name="psax", bufs=1,
                                              space="PSUM") as psax:
                                nc.sync.dma_start(
                                    out=xts,
                                    in_=xt_ext.rearrange("(a p) t -> p a t",
                                                         p=128))
                                nc.sync.dma_start(
                                    out=xres,
                                    in_=xtq_ext.rearrange("(a p) t -> p a t",
                                                          p=128))
                                rstd = pax.tile([1, T], BF16, tag="rstd",
                                                bufs=1)
                                colnorm_stats(xts, T, pax, psax, rstd)
                                rstdq = pax.tile([1, CH], BF16, tag="rstdq",
                                                 bufs=1)
                                colnorm_stats(xres, CH, pax, psax, rstdq)
                                # broadcast rstd across partitions
                                for ci in range(4):
                                    pb = psax.tile([128, 512], F32,
                                                   tag="bcast", bufs=2,
                                                   name="bcast")
                                    csl = slice(ci * 512, (ci + 1) * 512)
                                    nc.tensor.matmul(pb, ones_row,
                                                     rstd[:, csl],
                                                     start=True, stop=True)
                                    nc.vector.tensor_copy(rb_sb[:, csl], pb)
                                pbq = psax.tile([128, CH], F32, tag="bcast",
                                                bufs=2, name="pbq")
                                nc.tensor.matmul(pbq, ones_row, rstdq,
                                                 start=True, stop=True)
                                nc.vector.tensor_copy(rbq_sb, pbq)
                                # rstdT[p, st] = rstd[128*st + p] via PE
                                # transposes of [1,128] row slices
                                for st in range(TT):
                                    pt = psax.tile([128, 1], BF16, tag="ptr",
                                                   bufs=2, name="ptr")
                                    nc.tensor.transpose(
                                        pt,
                                        rstd[:, st * 128:(st + 1) * 128],
                                        ident[:1, :1])
                                    nc.vector.tensor_copy(
                                        rstdT[:, st:st + 1], pt)
                                if DBG:
                                    nc.sync.dma_start(out=dbg["rstd"],
                                                      in_=rstd)
                                    nc.sync.dma_start(out=dbg["rbq"],
                                                      in_=rbq_sb)
                                    nc.sync.dma_start(out=dbg["rstdT"],
                                                      in_=rstdT)

                            # ---- Phase B: projections + rope + norm-fold
                            with ExitStack() as pb_sc:
                                pb_ = pb_sc.enter_context(
                                    tc.tile_pool(name="pb", bufs=2))
                                psb = pb_sc.enter_context(
                                    tc.tile_pool(name="psb", bufs=2,
                                                 space="PSUM"))
                                sin_sb = pb_.tile([128, T], BF16, tag="sin",
                                                  bufs=1)
                                nc.sync.dma_start(out=sin_sb, in_=sin_ext)
                                cos_sb = pb_.tile([128, T], BF16, tag="cos",
                                                  bufs=1)
                                nc.sync.dma_start(out=cos_sb, in_=cos_ext)
                                sinq_sb = pb_.tile([128, CH], BF16,
                                                   tag="sinq", bufs=1)
                                nc.sync.dma_start(out=sinq_sb, in_=sinq_ext)
                                cosq_sb = pb_.tile([128, CH], BF16,
                                                   tag="cosq", bufs=1)
                                nc.sync.dma_start(out=cosq_sb, in_=cosq_ext)
                                wkv_sb = pb_.tile([128, DT, 2 * H], BF16,
                                                  tag="wkv", bufs=1)
                                nc.sync.dma_start(out=wkv_sb, in_=wkv_ext)

                                def rope_scale(dst1, dst2, p1, p2, cs, sn,
                                               rb, ncols):
                                    """dst = rope(p1,p2) * rb (norm fold)."""
                                    t1 = pb_.tile([128, ncols], BF16,
                                                  tag="rp1")
                                    t2 = pb_.tile([128, ncols], BF16,
                                                  tag="rp2")
                                    nc.vector.tensor_tensor(t1, p1, cs,
                                                            op=ALU.mult)
                                    nc.vector.tensor_tensor(t2, p2, sn,
                                                            op=ALU.mult)
                                    nc.vector.tensor_tensor(t1, t1, t2,
                                                            op=ALU.subtract)
                                    nc.vector.tensor_tensor(dst1, t1, rb,
                                                            op=ALU.mult)
                                    nc.vector.tensor_tensor(t1, p2, cs,
                                                            op=ALU.mult)
                                    nc.vector.tensor_tensor(t2, p1, sn,
                                                            op=ALU.mult)
                                    nc.vector.tensor_tensor(t1, t1, t2,
                                                            op=ALU.add)
                                    nc.vector.tensor_tensor(dst2, t1, rb,
                                                            op=ALU.mult)

                                # k (full T) + rope + rstd fold
                                for ci in range(4):
                                    csl = slice(ci * 512, (ci + 1) * 512)
                                    p1 = psb.tile([128, 512], F32, tag="p1")
                                    p2 = psb.tile([128, 512], F32, tag="p2")
                                    for kd in range(DT):
                                        nc.tensor.matmul(
                                            p1, wkv_sb[:, kd, 0:128],
                                            xts[:, kd, csl],
                                            start=kd == 0, stop=kd == DT - 1)
                                    for kd in range(DT):
                                        nc.tensor.matmul(
                                            p2, wkv_sb[:, kd, 128:256],
                                            xts[:, kd, csl],
                                            start=kd == 0, stop=kd == DT - 1)
                                    rope_scale(kT[:, 0, csl], kT[:, 1, csl],
                                               p1, p2, cos_sb[:, csl],
                                               sin_sb[:, csl],
                                               rb_sb[:, csl], 512)
                                # v natural [s, h], rstd fold per partition
                                for st in range(TT):
                                    pv = psb.tile([128, H], F32, tag="pv")
                                    for kd in range(DT):
                                        nc.tensor.matmul(
                                            pv,
                                            xts[:, kd,
                                                st * 128:(st + 1) * 128],
                                            wkv_sb[:, kd, H:2 * H],
                                            start=kd == 0, stop=kd == DT - 1)
                                    nc.vector.tensor_scalar_mul(
                                        v_sb[:, st], pv, rstdT[:, st:st + 1])
                                # q (chunk) + rope + rstdq fold
                                for n in range(NH):
                                    p1 = psb.tile([128, CH], F32, tag="p1")
                                    p2 = psb.tile([128, CH], F32, tag="p2")
                                    for j, ph in ((0, p1), (1, p2)):
                                        hb = 2 * n + j
                                        wqt = pb_.tile([128, DT, 128], BF16,
                                                       tag="wqt", bufs=2)
                                        nc.sync.dma_start(out=wqt,
                                                          in_=wq_ext[hb])
                                        for kd in range(DT):
                                            nc.tensor.matmul(
                                                ph, wqt[:, kd], xres[:, kd],
                                                start=kd == 0,
                                                stop=kd == DT - 1)
                                    rope_scale(qT[:, 2 * n], qT[:, 2 * n + 1],
                                               p1, p2, cosq_sb, sinq_sb,
                                               rbq_sb, CH)
                                if DBG:
                                    nc.sync.dma_start(out=dbg["kT"], in_=kT)
                                    nc.sync.dma_start(out=dbg["v"], in_=v_sb)
                                    nc.sync.dma_start(out=dbg["qT"], in_=qT)

                        # ---- Phase C: attention (transposed logits) ----
                        with ExitStack() as pc_sc:
                            pc = pc_sc.enter_context(
                                tc.tile_pool(name="pc", bufs=2))
                            psc = pc_sc.enter_context(
                                tc.tile_pool(name="psc", bufs=2,
                                             space="PSUM"))
                            mm_sb = pc.tile([128, TT, CH], BF16, tag="mm",
                                            bufs=1)
                            nc.sync.dma_start(out=mm_sb, in_=mm_ext)
                            for n in range(NH):
                                explT = pc.tile([128, TT, CH], BF16,
                                                tag="explT")
                                rsum = psc.tile([1, CH], F32, tag="rsum")
                                for st in range(TT):
                                    ssl = slice(st * 128, (st + 1) * 128)
                                    pl = psc.tile([128, CH], F32, tag="pl")
                                    nc.tensor.matmul(pl, kT[:, 0, ssl],
                                                     qT[:, 2 * n],
                                                     start=True, stop=False)
                                    nc.tensor.matmul(pl, kT[:, 1, ssl],
                                                     qT[:, 2 * n + 1],
                                                     start=False, stop=True)
                                    nc.scalar.activation(out=explT[:, st],
                                                         in_=pl, func=AF.Exp,
                                                         bias=ebias)
                                    nc.vector.tensor_tensor(explT[:, st],
                                                            explT[:, st],
                                                            mm_sb[:, st],
                                                            op=ALU.mult)
                                    nc.tensor.matmul(rsum, ones,
                                                     explT[:, st],
                                                     start=st == 0,
                                                     stop=st == TT - 1)
                                rrec = pc.tile([1, CH], BF16, tag="rrec")
                                with nc.allow_low_precision(
                                        reason="softmax 1/sum bf16"):
                                    nc.vector.reciprocal(out=rrec, in_=rsum)
                                rb = psc.tile([128, CH], F32, tag="rb")
                                nc.tensor.matmul(rb, ones_row, rrec,
                                                 start=True, stop=True)
                                rb_c = pc.tile([128, CH], BF16, tag="rbc")
                                nc.vector.tensor_copy(rb_c, rb)
                                for m in range(2):
                                    pe_ = psc.tile([128, CH], F32, tag="enc")
                                    for st in range(TT):
                                        nc.tensor.matmul(
                                            pe_,
                                            v_sb[:, st,
                                                 m * 128:(m + 1) * 128],
                                            explT[:, st],
                                            start=st == 0, stop=st == TT - 1)
                                    nc.vector.tensor_tensor(
                                        encA[:, 2 * n + m], pe_, rb_c,
                                        op=ALU.mult)
                                if DBG and n == 0:
                                    nc.sync.dma_start(out=dbg["expl"],
                                                      in_=explT)
                            if DBG:
                                nc.sync.dma_start(out=dbg["encA"], in_=encA)

                    # ---- Phase D: o-proj + residual + ffn-norm ----
                    with ExitStack() as pd_sc:
                        pd = pd_sc.enter_context(
                            tc.tile_pool(name="pd", bufs=2))
                        with tc.tile_pool(name="psdo", bufs=1,
                                          space="PSUM") as psdo:
                            for half in range(2):
                                aps = [psdo.tile([128, CH], F32,
                                                 tag=f"ao{i}", bufs=1,
                                                 name=f"ao{i}")
                                       for i in range(8)]
                                for hb in range(HB):
                                    wot = pd.tile([128, 1024], BF16,
                                                  tag="wot", bufs=3)
                                    nc.sync.dma_start(
                                        out=wot,
                                        in_=wo_ext[hb][:, half * 1024:
                                                       (half + 1) * 1024])
                                    for i in range(8):
                                        nc.tensor.matmul(
                                            aps[i],
                                            wot[:, i * 128:(i + 1) * 128],
                                            encA[:, hb],
                                            start=hb == 0, stop=hb == HB - 1)
                                for i in range(8):
                                    kd = half * 8 + i
                                    nc.vector.tensor_tensor(
                                        x2c[:, kd], aps[i], xres[:, kd],
                                        op=ALU.add)
                        # ffn norm on x2c columns
                        with tc.tile_pool(name="psd2", bufs=1,
                                          space="PSUM") as psd2:
                            ssq2 = psd2.tile([1, CH], F32, tag="ssq2",
                                             bufs=1)
                            for kd in range(DT):
                                sq2 = pd.tile([128, CH], BF16, tag="sq2")
                                nc.scalar.activation(out=sq2, in_=x2c[:, kd],
                                                     func=AF.Square)
                                nc.tensor.matmul(ssq2, ones, sq2,
                                                 start=kd == 0,
                                                 stop=kd == DT - 1)
                            std2 = pd.tile([1, CH], F32, tag="std2", bufs=1)
                            nc.scalar.activation(out=std2, in_=ssq2,
                                                 func=AF.Sqrt, bias=eps,
                                                 scale=1.0 / D)
                            rstd2 = pd.tile([1, CH], BF16, tag="rstd2",
                                            bufs=1)
                            with nc.allow_low_precision(
                                    reason="rstd in bf16 by design"):
                                nc.vector.reciprocal(out=rstd2, in_=std2)
                            rb2 = psd2.tile([128, CH], F32, tag="rb2",
                                            bufs=1)
                            nc.tensor.matmul(rb2, ones_row, rstd2,
                                             start=True, stop=True)
                            rb2_sb = pd.tile([128, CH], BF16, tag="rb2sb",
                                             bufs=1)
                            nc.vector.tensor_copy(rb2_sb, rb2)
                            for kd in range(DT):
                                nc.vector.tensor_tensor(h2c[:, kd],
                                                        x2c[:, kd], rb2_sb,
                                                        op=ALU.mult)
                        if DBG:
                            nc.sync.dma_start(out=dbg["x2c"], in_=x2c)
                            nc.sync.dma_start(out=dbg["h2c"], in_=h2c)

                # ---- Phase E: MLP (two F-halves), output ----
                with ExitStack() as pe_sc:
                    pe = pe_sc.enter_context(tc.tile_pool(name="pe", bufs=2))
                    pse = pe_sc.enter_context(
                        tc.tile_pool(name="pse", bufs=2, space="PSUM"))
                    downA = pe.tile([128, DT, CH], BF16, tag="downA", bufs=1)
                    for half in range(2):
                        ffT = pe.tile([128, FH, CH], BF16, tag="ffT", bufs=1)
                        for fi in range(FH):
                            fb = half * FH + fi
                            wgf = pe.tile([128, DT, 256], BF16, tag="wgf",
                                          bufs=3)
                            nc.sync.dma_start(out=wgf, in_=wg_ext[fb])
                            gps = pse.tile([128, CH], F32, tag="gps")
                            ups = pse.tile([128, CH], F32, tag="ups")
                            for kd in range(DT):
                                nc.tensor.matmul(gps, wgf[:, kd, 0:128],
                                                 h2c[:, kd],
                                                 start=kd == 0,
                                                 stop=kd == DT - 1)
                            for kd in range(DT):
                                nc.tensor.matmul(ups, wgf[:, kd, 128:256],
                                                 h2c[:, kd],
                                                 start=kd == 0,
                                                 stop=kd == DT - 1)
                            ga = pe.tile([128, CH], BF16, tag="ga")
                            nc.scalar.activation(out=ga, in_=gps,
                                                 func=AF.Gelu_apprx_tanh)
                            nc.vector.tensor_tensor(ffT[:, fi], ga, ups,
                                                    op=ALU.mult)
                        for kd in range(DT):
                            wlt = pe.tile([128, FH, 128], BF16, tag="wlt",
                                          bufs=2)
                            nc.sync.dma_start(
                                out=wlt,
                                in_=wl_ext[kd][:, half * FH:(half + 1) * FH])
                            dps = pse.tile([128, CH], F32, tag="dps")
                            for fi in range(FH):
                                nc.tensor.matmul(dps, wlt[:, fi], ffT[:, fi],
                                                 start=fi == 0,
                                                 stop=fi == FH - 1)
                            if half == 0:
                                nc.vector.tensor_copy(downA[:, kd], dps)
                            else:
                                ot = pe.tile([128, CH], F32, tag="ot",
                                             bufs=3)
                                nc.vector.tensor_tensor(ot, dps,
                                                        downA[:, kd],
                                                        op=ALU.add)
                                nc.vector.tensor_tensor(ot, ot, x2c[:, kd],
                                                        op=ALU.add)
                                nc.sync.dma_start(
                                    out=out_ext[kd * 128:(kd + 1) * 128],
                                    in_=ot)
    nc.compile()
    return nc


# ---------------------------------------------------------------------------
# host side
# ---------------------------------------------------------------------------

def make_in_maps(cfg, x, positions, attn_mask, scale_attn, w_q, w_kv, w_o,
                 scale_ffn, w_gating, w_linear):
    bf = ml_dtypes.bfloat16
    B = np.asarray(x).shape[0]
    s1a = (1.0 + np.asarray(scale_attn, np.float32))[:, None]
    s1f = (1.0 + np.asarray(scale_ffn, np.float32))[:, None]

    # weights (shared by every core)
    wq_f = (np.asarray(w_q, np.float32) * s1a[None] * H ** -0.5)  # [N, D, H]
    Wq2 = np.concatenate(list(wq_f), axis=1)                      # [D, N*H]
    wq_t = np.ascontiguousarray(
        Wq2.reshape(DT, 128, HB, 128).transpose(2, 1, 0, 3).astype(bf))
    k_w = np.asarray(w_kv[0, 0], np.float32) * s1a
    v_w = np.asarray(w_kv[1, 0], np.float32) * s1a
    wkv_t = np.ascontiguousarray(
        np.concatenate([k_w, v_w], axis=1).astype(bf)
        .reshape(DT, 128, 2 * H).transpose(1, 0, 2))
    Wo2 = np.asarray(w_o, np.float32).reshape(NH * H, D)
    wo_t = np.ascontiguousarray(Wo2.reshape(HB, 128, D).astype(bf))
    gate = (np.asarray(w_gating[0], np.float32) * s1f).astype(bf)
    up = (np.asarray(w_gating[1], np.float32) * s1f).astype(bf)
    gate = gate.reshape(DT, 128, FB, 128).transpose(2, 1, 0, 3)
    up = up.reshape(DT, 128, FB, 128).transpose(2, 1, 0, 3)
    wg_t = np.ascontiguousarray(np.concatenate([gate, up], axis=3))
    wl_t = np.ascontiguousarray(
        np.asarray(w_linear, np.float32).astype(bf)
        .reshape(FB, 128, DT, 128).transpose(2, 1, 0, 3))

    freq = 10000.0 ** (2.0 / H * np.arange(H // 2, dtype=np.float32))
    mask = np.asarray(attn_mask)  # [B, 1, T, T] bool
    in_maps = []
    for c in range(8):
        b, r = divmod(c, 4)
        b = min(b, B - 1)
        xT = np.ascontiguousarray(
            np.asarray(x[b], np.float32).T.astype(bf))          # [D, T]
        pos = np.asarray(positions[b], np.float32)
        rad = pos[None, :] / freq[:, None]                       # [H/2, T]
        csl = slice(r * CH, (r + 1) * CH)
        # mmask[p, st, t] = mask[b, 0, chunk_t, s=128*st+p]
        mchunk = mask[b, 0, csl, :]                              # [CH, S]
        mm = np.ascontiguousarray(
            mchunk.T.reshape(TT, 128, CH).transpose(1, 0, 2)
            .astype(bf))
        in_maps.append({
            "xt": xT,
            "xtq": np.ascontiguousarray(xT[:, csl]),
            "wq": wq_t, "wkv": wkv_t, "wo": wo_t, "wg": wg_t, "wl": wl_t,
            "sin": np.ascontiguousarray(np.sin(rad).astype(bf)),
            "cos": np.ascontiguousarray(np.cos(rad).astype(bf)),
            "sinq": np.ascontiguousarray(np.sin(rad[:, csl]).astype(bf)),
            "cosq": np.ascontiguousarray(np.cos(rad[:, csl]).astype(bf)),
            "mmask": mm,
        })
    return in_maps


def assemble(cfg, results, B):
    out = np.empty((B, T, D), np.float32)
    for c in range(8):
        b, r = divmod(c, 4)
        if b >= B:
            continue
        out[b, r * CH:(r + 1) * CH, :] = results[c]["out"].T
    return out


# cached compiled program + jitted runner -----------------------------------

_CACHE = {}


def _get_runner(cfg_key, cfg):
    if cfg_key in _CACHE:
        return _CACHE[cfg_key]
    runner = _runner_from_nc(build(cfg))
    _CACHE[cfg_key] = runner
    return runner


def _runner_from_nc(nc):
    import jax
    from jax.experimental.shard_map import shard_map
    from jax.sharding import Mesh, PartitionSpec
    from concourse import bass2jax

    bass2jax.install_neuronx_cc_hook()

    partition_name = (nc.partition_id_tensor.name
                      if nc.partition_id_tensor else None)
    in_names, out_names, out_avals, zero_shapes = [], [], [], []
    for alloc in nc.m.functions[0].allocations:
        if not isinstance(alloc, mybir.MemoryLocationSet):
            continue
        name = alloc.memorylocations[0].name
        if alloc.kind == "ExternalInput":
            if name != partition_name:
                in_names.append(name)
        elif alloc.kind == "ExternalOutput":
            out_names.append(name)
            shape = tuple(alloc.tensor_shape)
            dtype = mybir.dt.np(alloc.dtype)
            out_avals.append(jax.core.ShapedArray(shape, dtype))
            zero_shapes.append((shape, dtype))
    n_params = len(in_names)
    all_in_names = in_names + out_names
    if partition_name is not None:
        all_in_names = all_in_names + [partition_name]

    def _body(*args):
        operands = list(args)
        if partition_name is not None:
            operands.append(bass2jax.partition_id_tensor())
        outs = bass2jax._bass_exec_p.bind(
            *operands,
            out_avals=tuple(out_avals),
            in_names=tuple(all_in_names),
            out_names=tuple(out_names),
            lowering_input_output_aliases=(),
            sim_require_finite=True,
            sim_require_nnan=True,
            nc=nc,
        )
        return tuple(outs)

    n_outs = len(out_names)
    donate = tuple(range(n_params, n_params + n_outs))
    devices = jax.devices()[:8]
    mesh = Mesh(np.asarray(devices), ("core",))
    in_specs = (PartitionSpec("core"),) * (n_params + n_outs)
    out_specs = (PartitionSpec("core"),) * n_outs
    sharded = jax.jit(
        shard_map(_body, mesh=mesh, in_specs=in_specs, out_specs=out_specs,
                  check_rep=False),
        donate_argnums=donate, keep_unused=True)

    class Runner:
        pass

    runner = Runner()
    runner.sharded = sharded
    runner.nc = nc
    runner.mesh = mesh
    runner.in_names = in_names
    runner.out_names = out_names
    runner.out_avals = out_avals
    runner.zero_shapes = zero_shapes

    def concat_inputs(in_maps):
        return [np.concatenate([np.asarray(m[name]) for m in in_maps],
                               axis=0) for name in in_names]

    def make_zeros():
        return [np.zeros((8 * s[0], *s[1:]), d) for s, d in zero_shapes]

    def split_outputs(out_arrs):
        return [
            {name: np.asarray(out_arrs[i]).reshape(8, *out_avals[i].shape)[c]
             for i, name in enumerate(out_names)}
            for c in range(8)
        ]

    runner.concat_inputs = concat_inputs
    runner.make_zeros = make_zeros
    runner.split_outputs = split_outputs

    def run(in_maps):
        out_arrs = sharded(*concat_inputs(in_maps), *make_zeros())
        return split_outputs(out_arrs)

    runner.run = run

    def make_fast(arg_arrays):
        """AOT-compile the no-donate fast-dispatch variant for the given
        device-resident args (inputs followed by output buffers)."""
        specs = [jax.ShapeDtypeStruct(a.shape, a.dtype, sharding=a.sharding)
                 for a in arg_arrays]
        return bass2jax.fast_dispatch_compile(
            lambda: jax.jit(
                shard_map(_body, mesh=mesh, in_specs=in_specs,
                          out_specs=out_specs, check_rep=False),
                keep_unused=True).lower(*specs).compile())

    runner.make_fast = make_fast
    return runner


def run_cfg(cfg, inputs):
    cfg_key = tuple(sorted(cfg.items()))
    runner = _get_runner(cfg_key, cfg)
    in_maps = make_in_maps(cfg, **inputs)
    results = runner.run(in_maps)
    return assemble(cfg, results, np.asarray(inputs["x"]).shape[0])


def kernel(**inputs):
    return run_cfg(FULL_CFG, inputs)


# revision 5
# speedup vs baseline: 1.2132x; 1.1863x over previous
"""Trainium2 Bass kernel v3 for dense transformer block nn_Block_68221260529679.

Layout: B=2, T=2048, D=2048, N=8 q-heads, K=1 kv-head, H=256, F=16384.

Sharding (8 NeuronCores): DP over batch (2 groups of 4) x T-split within the
group (4 chunks of 512 tokens).  Core c = 4*b + r handles batch b, tokens
[512r, 512r+512).  Every core computes the full k/v projection for its batch
(K=1 kv-head, cheap) and the full attention + MLP for its own 512 tokens with
the FULL weights.  Zero collectives; one SPMD program with no rank-dependent
control flow — all rank variation is carried by the input data (token chunk,
rope tables for the chunk, attention mask tiles).

Everything on device lives in transposed [feature, token] layout, so there are
no PE transposes at all:
  - x^T arrives from host (bf16); rmsnorm per-token stats are computed with
    ones-vector matmuls on the PE and broadcast back with a rank-1 matmul.
  - logits are computed transposed (partition = key, free = query); softmax
    normalization is deferred: exp(l - 8) without row-max, masked by a 0/1
    data mask, summed per query with a ones-matmul, and the reciprocal is
    folded into the attention-vector output columns.
  - o-proj, residuals, ffn-norm, gate/up/gelu and down-proj all stay in
    [feature, token] form; the host transposes the [D, 512] output back.

All matmuls bf16 with fp32 PSUM accumulation.  MLP runs in two F-halves so
the intermediate ff^T fits SBUF (64 KiB/partition per half).
"""

from contextlib import ExitStack

import numpy as np
import ml_dtypes

import concourse.bass as bass
import concourse.mybir as mybir
import concourse.tile as tile
from concourse import bacc
from concourse.masks import make_identity

F32 = mybir.dt.float32
BF16 = mybir.dt.bfloat16
AF = mybir.ActivationFunctionType
ALU = mybir.AluOpType

T, D, H, NH, F = 2048, 2048, 256, 8, 16384
CH = 512                 # tokens per core
DT = D // 128            # 16 d-blocks
TT = T // 128            # 16 t/s-tiles
HB = NH * H // 128       # 16 enc blocks
FB = F // 128            # 128 f-blocks
FH = FB // 2             # f-blocks per MLP pass
EXP_BIAS = -8.0          # folded stabilizer: exp(l - 8)

FULL_CFG = dict(version=3)


def build(cfg):
    REPS = cfg.get("reps", 1)
    nc = bacc.Bacc("TRN2", target_bir_lowering=False, debug=False,
                   num_devices=8)
    xt_ext = nc.dram_tensor("xt", [D, T], BF16, kind="ExternalInput").ap()
    xtq_ext = nc.dram_tensor("xtq", [D, CH], BF16, kind="ExternalInput").ap()
    wq_ext = nc.dram_tensor("wq", [HB, 128, DT, 128], BF16,
                            kind="ExternalInput").ap()
    wkv_ext = nc.dram_tensor("wkv", [128, DT, 2 * H], BF16,
                             kind="ExternalInput").ap()
    wo_ext = nc.dram_tensor("wo", [HB, 128, D], BF16,
                            kind="ExternalInput").ap()
    wg_ext = nc.dram_tensor("wg", [FB, 128, DT, 256], BF16,
                            kind="ExternalInput").ap()
    wl_ext = nc.dram_tensor("wl", [DT, 128, FB, 128], BF16,
                            kind="ExternalInput").ap()
    sin_ext = nc.dram_tensor("sin", [128, T], BF16, kind="ExternalInput").ap()
    cos_ext = nc.dram_tensor("cos", [128, T], BF16, kind="ExternalInput").ap()
    sinq_ext = nc.dram_tensor("sinq", [128, CH], BF16,
                              kind="ExternalInput").ap()
    cosq_ext = nc.dram_tensor("cosq", [128, CH], BF16,
                              kind="ExternalInput").ap()
    mm_ext = nc.dram_tensor("mmask", [128, TT, CH], BF16,
                            kind="ExternalInput").ap()
    out_ext = nc.dram_tensor("out", [D, CH], F32, kind="ExternalOutput").ap()
    DBG = cfg.get("debug", False)
    if DBG:
        dbg = {k: nc.dram_tensor(f"d_{k}", shp, dt, kind="ExternalOutput").ap()
               for k, shp, dt in [
                   ("rstd", [1, T], BF16), ("rbq", [128, CH], BF16),
                   ("rstdT", [128, TT], F32), ("kT", [128, 2, T], BF16),
                   ("v", [128, TT, H], BF16), ("qT", [128, HB, CH], BF16),
                   ("expl", [128, TT, CH], BF16),
                   ("encA", [128, HB, CH], BF16),
                   ("x2c", [128, DT, CH], BF16),
                   ("h2c", [128, DT, CH], BF16)]}

    with tile.TileContext(nc) as tc, ExitStack() as top:
        cons = top.enter_context(tc.tile_pool(name="cons", bufs=1))
        ones = cons.tile([128, 1], BF16)
        nc.vector.memset(ones, 1.0)
        ones_row = cons.tile([1, 128], BF16)
        nc.vector.memset(ones_row, 1.0)
        eps = cons.tile([1, 1], F32)
        nc.vector.memset(eps, 1e-6)
        ebias = cons.tile([128, 1], F32)
        nc.vector.memset(ebias, EXP_BIAS)
        ident = cons.tile([128, 128], BF16)
        make_identity(nc, ident)

        def colnorm_stats(src, ncols, p_sq, p_ps, out_bf):
            """rstd (bf16, [1, ncols]) for columns of src [128, DT, ncols]."""
            nchunk = ncols // 512
            ssqs = [p_ps.tile([1, 512], F32, tag=f"ssq{i}", bufs=1,
                              name=f"ssq{i}") for i in range(nchunk)]
            for kd in range(DT):
                sq = p_sq.tile([128, ncols], BF16, tag="sq")
                nc.scalar.activation(out=sq, in_=src[:, kd], func=AF.Square)
                for ci in range(nchunk):
                    nc.tensor.matmul(ssqs[ci], ones,
                                     sq[:, ci * 512:(ci + 1) * 512],
                                     start=kd == 0, stop=kd == DT - 1)
            for ci in range(nchunk):
                std = p_sq.tile([1, 512], F32, tag="std", name="std")
                nc.scalar.activation(out=std, in_=ssqs[ci], func=AF.Sqrt,
                                     bias=eps, scale=1.0 / D)
                with nc.allow_low_precision(reason="rstd in bf16 by design"):
                    nc.vector.reciprocal(
                        out=out_bf[:, ci * 512:(ci + 1) * 512], in_=std)

        for _rep in range(REPS):
            with ExitStack() as rep_sc:
                # long-lived pools, nested by live range (LIFO close order)
                p_x2 = rep_sc.enter_context(tc.tile_pool(name="p_x2",
                                                         bufs=1))
                x2c = p_x2.tile([128, DT, CH], BF16, tag="x2c")
                h2c = p_x2.tile([128, DT, CH], BF16, tag="h2c")

                with ExitStack() as res_sc:
                    p_res = res_sc.enter_context(
                        tc.tile_pool(name="p_res", bufs=1))
                    xres = p_res.tile([128, DT, CH], BF16, tag="xres")
                    encA = p_res.tile([128, HB, CH], BF16, tag="encA")

                    with ExitStack() as qkv_sc:
                        p_qkv = qkv_sc.enter_context(
                            tc.tile_pool(name="p_qkv", bufs=1))
                        qT = p_qkv.tile([128, HB, CH], BF16, tag="qT")
                        kT = p_qkv.tile([128, 2, T], BF16, tag="kT")
                        v_sb = p_qkv.tile([128, TT, H], BF16, tag="v")
                        rstdT = p_qkv.tile([128, TT], F32, tag="rstdT")

                        with ExitStack() as ab_sc:
                            p_ax = ab_sc.enter_context(
                                tc.tile_pool(name="p_ax", bufs=1))
                            xts = p_ax.tile([128, DT, T], BF16, tag="xts")
                            rb_sb = p_ax.tile([128, T], BF16, tag="rbsb")
                            rbq_sb = p_ax.tile([128, CH], BF16, tag="rbqsb")

                            # ---- Phase A: load x^T, column rmsnorm stats
                            with tc.tile_pool(name="pax", bufs=2) as pax, \
                                 tc.tile_pool(name="psax", bufs=1,
                                              space="PSUM") as psax:
                                xt_r = xt_ext.rearrange(
                                    "(a p) t -> p a t", p=128)
                                for kd in range(DT):
                                    nc.sync.dma_start(
                                        out=xts[:, kd], in_=xt_r[:, kd])
                                nc.sync.dma_start(
                                    out=xres,
                                    in_=xtq_ext.rearrange("(a p) t -> p a t",
                                                          p=128))
                                rstd = pax.tile([1, T], BF16, tag="rstd",
                                                bufs=1)
                                colnorm_stats(xts, T, pax, psax, rstd)
                                rstdq = pax.tile([1, CH], BF16, tag="rstdq",
                                                 bufs=1)
                                colnorm_stats(xres, CH, pax, psax, rstdq)
                                # broadcast rstd across partitions
                                for ci in range(4):
                                    pb = psax.tile([128, 512], F32,
                                                   tag="bcast", bufs=2,
                                                   name="bcast")
                                    csl = slice(ci * 512, (ci + 1) * 512)
                                    nc.tensor.matmul(pb, ones_row,
                                                     rstd[:, csl],
                                                     start=True, stop=True)
                                    nc.vector.tensor_copy(rb_sb[:, csl], pb)
                                pbq = psax.tile([128, CH], F32, tag="bcast",
                                                bufs=2, name="pbq")
                                nc.tensor.matmul(pbq, ones_row, rstdq,
                                                 start=True, stop=True)
                                nc.vector.tensor_copy(rbq_sb, pbq)
                                # rstdT[p, st] = rstd[128*st + p] via PE
                                # transposes of [1,128] row slices
                                for st in range(TT):
                                    pt = psax.tile([128, 1], BF16, tag="ptr",
                                                   bufs=2, name="ptr")
                                    nc.tensor.transpose(
                                        pt,
                                        rstd[:, st * 128:(st + 1) * 128],
                                        ident[:1, :1])
                                    nc.vector.tensor_copy(
                                        rstdT[:, st:st + 1], pt)
                                if DBG:
                                    nc.sync.dma_start(out=dbg["rstd"],
                                                      in_=rstd)
                                    nc.sync.dma_start(out=dbg["rbq"],
                                                      in_=rbq_sb)
                                    nc.sync.dma_start(out=dbg["rstdT"],
                                                      in_=rstdT)

                            # ---- Phase B: projections + rope + norm-fold
                            with ExitStack() as pb_sc:
                                pb_ = pb_sc.enter_context(
                                    tc.tile_pool(name="pb", bufs=2))
                                psb = pb_sc.enter_context(
                                    tc.tile_pool(name="psb", bufs=2,
                                                 space="PSUM"))
                                sin_sb = pb_.tile([128, T], BF16, tag="sin",
                                                  bufs=1)
                                nc.sync.dma_start(out=sin_sb, in_=sin_ext)
                                cos_sb = pb_.tile([128, T], BF16, tag="cos",
                                                  bufs=1)
                                nc.sync.dma_start(out=cos_sb, in_=cos_ext)
                                sinq_sb = pb_.tile([128, CH], BF16,
                                                   tag="sinq", bufs=1)
                                nc.sync.dma_start(out=sinq_sb, in_=sinq_ext)
                                cosq_sb = pb_.tile([128, CH], BF16,
                                                   tag="cosq", bufs=1)
                                nc.sync.dma_start(out=cosq_sb, in_=cosq_ext)
                                wkv_sb = pb_.tile([128, DT, 2 * H], BF16,
                                                  tag="wkv", bufs=1)
                                nc.sync.dma_start(out=wkv_sb, in_=wkv_ext)

                                def rope_scale(dst1, dst2, p1, p2, cs, sn,
                                               rb, ncols):
                                    """dst = rope(p1,p2) * rb (norm fold)."""
                                    t1 = pb_.tile([128, ncols], BF16,
                                                  tag="rp1")
                                    t2 = pb_.tile([128, ncols], BF16,
                                                  tag="rp2")
                                    nc.vector.tensor_tensor(t1, p1, cs,
                                                            op=ALU.mult)
                                    nc.vector.tensor_tensor(t2, p2, sn,
                                                            op=ALU.mult)
                                    nc.vector.tensor_tensor(t1, t1, t2,
                                                            op=ALU.subtract)
                                    nc.vector.tensor_tensor(dst1, t1, rb,
                                                            op=ALU.mult)
                                    nc.vector.tensor_tensor(t1, p2, cs,
                                                            op=ALU.mult)
                                    nc.vector.tensor_tensor(t2, p1, sn,
                                                            op=ALU.mult)
                                    nc.vector.tensor_tensor(t1, t1, t2,
                                                            op=ALU.add)
                                    nc.vector.tensor_tensor(dst2, t1, rb,
                                                            op=ALU.mult)

                                # k (full T) + rope + rstd fold
                                for ci in range(4):
                                    csl = slice(ci * 512, (ci + 1) * 512)
                                    p1 = psb.tile([128, 512], F32, tag="p1")
                                    p2 = psb.tile([128, 512], F32, tag="p2")
                                    for kd in range(DT):
                                        nc.tensor.matmul(
                                            p1, wkv_sb[:, kd, 0:128],
                                            xts[:, kd, csl],
                                            start=kd == 0, stop=kd == DT - 1)
                                    for kd in range(DT):
                                        nc.tensor.matmul(
                                            p2, wkv_sb[:, kd, 128:256],
                                            xts[:, kd, csl],
                                            start=kd == 0, stop=kd == DT - 1)
                                    rope_scale(kT[:, 0, csl], kT[:, 1, csl],
                                               p1, p2, cos_sb[:, csl],
                                               sin_sb[:, csl],
                                               rb_sb[:, csl], 512)
                                # v natural [s, h], rstd fold per partition
                                for st in range(TT):
                                    pv = psb.tile([128, H], F32, tag="pv")
                                    for kd in range(DT):
                                        nc.tensor.matmul(
                                            pv,
                                            xts[:, kd,
                                                st * 128:(st + 1) * 128],
                                            wkv_sb[:, kd, H:2 * H],
                                            start=kd == 0, stop=kd == DT - 1)
                                    nc.vector.tensor_scalar_mul(
                                        v_sb[:, st], pv, rstdT[:, st:st + 1])
                                # q (chunk) + rope + rstdq fold
                                for n in range(NH):
                                    p1 = psb.tile([128, CH], F32, tag="p1")
                                    p2 = psb.tile([128, CH], F32, tag="p2")
                                    for j, ph in ((0, p1), (1, p2)):
                                        hb = 2 * n + j
                                        wqt = pb_.tile([128, DT, 128], BF16,
                                                       tag="wqt", bufs=2)
                                        nc.sync.dma_start(out=wqt,
                                                          in_=wq_ext[hb])
                                        for kd in range(DT):
                                            nc.tensor.matmul(
                                                ph, wqt[:, kd], xres[:, kd],
                                                start=kd == 0,
                                                stop=kd == DT - 1)
                                    rope_scale(qT[:, 2 * n], qT[:, 2 * n + 1],
                                               p1, p2, cosq_sb, sinq_sb,
                                               rbq_sb, CH)
                                if DBG:
                                    nc.sync.dma_start(out=dbg["kT"], in_=kT)
                                    nc.sync.dma_start(out=dbg["v"], in_=v_sb)
                                    nc.sync.dma_start(out=dbg["qT"], in_=qT)

                        # ---- Phase C: attention (transposed logits) ----
                        with ExitStack() as pc_sc:
                            pc = pc_sc.enter_context(
                                tc.tile_pool(name="pc", bufs=2))
                            psc = pc_sc.enter_context(
                                tc.tile_pool(name="psc", bufs=2,
                                             space="PSUM"))
                            mm_sb = pc.tile([128, TT, CH], BF16, tag="mm",
                                            bufs=1)
                            nc.sync.dma_start(out=mm_sb, in_=mm_ext)
                            for n in range(NH):
                                explT = pc.tile([128, TT, CH], BF16,
                                                tag="explT")
                                rsum = psc.tile([1, CH], F32, tag="rsum")
                                for st in range(TT):
                                    ssl = slice(st * 128, (st + 1) * 128)
                                    pl = psc.tile([128, CH], F32, tag="pl")
                                    nc.tensor.matmul(pl, kT[:, 0, ssl],
                                                     qT[:, 2 * n],
                                                     start=True, stop=False)
                                    nc.tensor.matmul(pl, kT[:, 1, ssl],
                                                     qT[:, 2 * n + 1],
                                                     start=False, stop=True)
                                    nc.scalar.activation(out=explT[:, st],
                                                         in_=pl, func=AF.Exp,
                                                         bias=ebias)
                                    nc.vector.tensor_tensor(explT[:, st],
                                                            explT[:, st],
                                                            mm_sb[:, st],
                                                            op=ALU.mult)
                                    nc.tensor.matmul(rsum, ones,
                                                     explT[:, st],
                                                     start=st == 0,
                                                     stop=st == TT - 1)
                                rrec = pc.tile([1, CH], BF16, tag="rrec")
                                with nc.allow_low_precision(
                                        reason="softmax 1/sum bf16"):
                                    nc.vector.reciprocal(out=rrec, in_=rsum)
                                rb = psc.tile([128, CH], F32, tag="rb")
                                nc.tensor.matmul(rb, ones_row, rrec,
                                                 start=True, stop=True)
                                rb_c = pc.tile([128, CH], BF16, tag="rbc")
                                nc.vector.tensor_copy(rb_c, rb)
                                for m in range(2):
                                    pe_ = psc.tile([128, CH], F32, tag="enc")
                                    for st in range(TT):
                                        nc.tensor.matmul(
                                            pe_,
                                            v_sb[:, st,
                                                 m * 128:(m + 1) * 128],
                                            explT[:, st],
                                            start=st == 0, stop=st == TT - 1)
                                    nc.vector.tensor_tensor(
                                        encA[:, 2 * n + m], pe_, rb_c,
                                        op=ALU.mult)
                                if DBG and n == 0:
                                    nc.sync.dma_start(out=dbg["expl"],
                                                      in_=explT)
                            if DBG:
                                nc.sync.dma_start(out=dbg["encA"], in_=encA)

                    # ---- Phase D: o-proj + residual + ffn-norm ----
                    with ExitStack() as pd_sc:
                        pd = pd_sc.enter_context(
                            tc.tile_pool(name="pd", bufs=2))
                        with tc.tile_pool(name="psdo", bufs=1,
                                          space="PSUM") as psdo:
                            for half in range(2):
                                aps = [psdo.tile([128, CH], F32,
                                                 tag=f"ao{i}", bufs=1,
                                                 name=f"ao{i}")
                                       for i in range(8)]
                                for hb in range(HB):
                                    wot = pd.tile([128, 1024], BF16,
                                                  tag="wot", bufs=3)
                                    nc.sync.dma_start(
                                        out=wot,
                                        in_=wo_ext[hb][:, half * 1024:
                                                       (half + 1) * 1024])
                                    for i in range(8):
                                        nc.tensor.matmul(
                                            aps[i],
                                            wot[:, i * 128:(i + 1) * 128],
                                            encA[:, hb],
                                            start=hb == 0, stop=hb == HB - 1)
                                for i in range(8):
                                    kd = half * 8 + i
                                    nc.vector.tensor_tensor(
                                        x2c[:, kd], aps[i], xres[:, kd],
                                        op=ALU.add)
                        # ffn norm on x2c columns
                        with tc.tile_pool(name="psd2", bufs=1,
                                          space="PSUM") as psd2:
                            ssq2 = psd2.tile([1, CH], F32, tag="ssq2",
                                             bufs=1)
                            for kd in range(DT):
                                sq2 = pd.tile([128, CH], BF16, tag="sq2")
                                nc.scalar.activation(out=sq2, in_=x2c[:, kd],
                                                     func=AF.Square)
                                nc.tensor.matmul(ssq2, ones, sq2,
                                                 start=kd == 0,
                                                 stop=kd == DT - 1)
                            std2 = pd.tile([1, CH], F32, tag="std2", bufs=1)
                            nc.scalar.activation(out=std2, in_=ssq2,
                                                 func=AF.Sqrt, bias=eps,
                                                 scale=1.0 / D)
                            rstd2 = pd.tile([1, CH], BF16, tag="rstd2",
                                            bufs=1)
                            with nc.allow_low_precision(
                                    reason="rstd in bf16 by design"):
                                nc.vector.reciprocal(out=rstd2, in_=std2)
                            rb2 = psd2.tile([128, CH], F32, tag="rb2",
                                            bufs=1)
                            nc.tensor.matmul(rb2, ones_row, rstd2,
                                             start=True, stop=True)
                            rb2_sb = pd.tile([128, CH], BF16, tag="rb2sb",
                                             bufs=1)
                            nc.vector.tensor_copy(rb2_sb, rb2)
                            for kd in range(DT):
                                nc.vector.tensor_tensor(h2c[:, kd],
                                                        x2c[:, kd], rb2_sb,
                                                        op=ALU.mult)
                        if DBG:
                            nc.sync.dma_start(out=dbg["x2c"], in_=x2c)
                            nc.sync.dma_start(out=dbg["h2c"], in_=h2c)

                # ---- Phase E: MLP (two F-halves), output ----
                with ExitStack() as pe_sc:
                    pe = pe_sc.enter_context(tc.tile_pool(name="pe", bufs=2))
                    pse = pe_sc.enter_context(
                        tc.tile_pool(name="pse", bufs=2, space="PSUM"))
                    downA = pe.tile([128, DT, CH], BF16, tag="downA", bufs=1)
                    for half in range(2):
                        ffT = pe.tile([128, FH, CH], BF16, tag="ffT", bufs=1)
                        for fi in range(FH):
                            fb = half * FH + fi
                            wgf = pe.tile([128, DT, 256], BF16, tag="wgf",
                                          bufs=3)
                            nc.sync.dma_start(out=wgf, in_=wg_ext[fb])
                            gps = pse.tile([128, CH], F32, tag="gps")
                            ups = pse.tile([128, CH], F32, tag="ups")
                            for kd in range(DT):
                                nc.tensor.matmul(gps, wgf[:, kd, 0:128],
                                                 h2c[:, kd],
                                                 start=kd == 0,
                                                 stop=kd == DT - 1)
                            for kd in range(DT):
                                nc.tensor.matmul(ups, wgf[:, kd, 128:256],
                                                 h2c[:, kd],
                                                 start=kd == 0,
                                                 stop=kd == DT - 1)
                            ga = pe.tile([128, CH], BF16, tag="ga")
                            nc.scalar.activation(out=ga, in_=gps,
                                                 func=AF.Gelu_apprx_tanh)
                            nc.vector.tensor_tensor(ffT[:, fi], ga, ups,
                                                    op=ALU.mult)
                        for kd in range(DT):
                            wlt = pe.tile([128, FH, 128], BF16, tag="wlt",
                                          bufs=2)
                            nc.sync.dma_start(
                                out=wlt,
                                in_=wl_ext[kd][:, half * FH:(half + 1) * FH])
                            dps = pse.tile([128, CH], F32, tag="dps")
                            for fi in range(FH):
                                nc.tensor.matmul(dps, wlt[:, fi], ffT[:, fi],
                                                 start=fi == 0,
                                                 stop=fi == FH - 1)
                            if half == 0:
                                nc.vector.tensor_copy(downA[:, kd], dps)
                            else:
                                ot = pe.tile([128, CH], F32, tag="ot",
                                             bufs=3)
                                nc.vector.tensor_tensor(ot, dps,
                                                        downA[:, kd],
                                                        op=ALU.add)
                                nc.vector.tensor_tensor(ot, ot, x2c[:, kd],
                                                        op=ALU.add)
                                nc.sync.dma_start(
                                    out=out_ext[kd * 128:(kd + 1) * 128],
                                    in_=ot)
    nc.compile()
    return nc


# ---------------------------------------------------------------------------
# host side
# ---------------------------------------------------------------------------

def make_in_maps(cfg, x, positions, attn_mask, scale_attn, w_q, w_kv, w_o,
                 scale_ffn, w_gating, w_linear):
    bf = ml_dtypes.bfloat16
    B = np.asarray(x).shape[0]
    s1a = (1.0 + np.asarray(scale_attn, np.float32))[:, None]
    s1f = (1.0 + np.asarray(scale_ffn, np.float32))[:, None]

    # weights (shared by every core)
    wq_f = (np.asarray(w_q, np.float32) * s1a[None] * H ** -0.5)  # [N, D, H]
    Wq2 = np.concatenate(list(wq_f), axis=1)                      # [D, N*H]
    wq_t = np.ascontiguousarray(
        Wq2.reshape(DT, 128, HB, 128).transpose(2, 1, 0, 3).astype(bf))
    k_w = np.asarray(w_kv[0, 0], np.float32) * s1a
    v_w = np.asarray(w_kv[1, 0], np.float32) * s1a
    wkv_t = np.ascontiguousarray(
        np.concatenate([k_w, v_w], axis=1).astype(bf)
        .reshape(DT, 128, 2 * H).transpose(1, 0, 2))
    Wo2 = np.asarray(w_o, np.float32).reshape(NH * H, D)
    wo_t = np.ascontiguousarray(Wo2.reshape(HB, 128, D).astype(bf))
    gate = (np.asarray(w_gating[0], np.float32) * s1f).astype(bf)
    up = (np.asarray(w_gating[1], np.float32) * s1f).astype(bf)
    gate = gate.reshape(DT, 128, FB, 128).transpose(2, 1, 0, 3)
    up = up.reshape(DT, 128, FB, 128).transpose(2, 1, 0, 3)
    wg_t = np.ascontiguousarray(np.concatenate([gate, up], axis=3))
    wl_t = np.ascontiguousarray(
        np.asarray(w_linear, np.float32).astype(bf)
        .reshape(FB, 128, DT, 128).transpose(2, 1, 0, 3))

    freq = 10000.0 ** (2.0 / H * np.arange(H // 2, dtype=np.float32))
    mask = np.asarray(attn_mask)  # [B, 1, T, T] bool
    in_maps = []
    for c in range(8):
        b, r = divmod(c, 4)
        b = min(b, B - 1)
        xT = np.ascontiguousarray(
            np.asarray(x[b], np.float32).T.astype(bf))          # [D, T]
        pos = np.asarray(positions[b], np.float32)
        rad = pos[None, :] / freq[:, None]                       # [H/2, T]
        csl = slice(r * CH, (r + 1) * CH)
        # mmask[p, st, t] = mask[b, 0, chunk_t, s=128*st+p]
        mchunk = mask[b, 0, csl, :]                              # [CH, S]
        mm = np.ascontiguousarray(
            mchunk.T.reshape(TT, 128, CH).transpose(1, 0, 2)
            .astype(bf))
        in_maps.append({
            "xt": xT,
            "xtq": np.ascontiguousarray(xT[:, csl]),
            "wq": wq_t, "wkv": wkv_t, "wo": wo_t, "wg": wg_t, "wl": wl_t,
            "sin": np.ascontiguousarray(np.sin(rad).astype(bf)),
            "cos": np.ascontiguousarray(np.cos(rad).astype(bf)),
            "sinq": np.ascontiguousarray(np.sin(rad[:, csl]).astype(bf)),
            "cosq": np.ascontiguousarray(np.cos(rad[:, csl]).astype(bf)),
            "mmask": mm,
        })
    return in_maps


def assemble(cfg, results, B):
    out = np.empty((B, T, D), np.float32)
    for c in range(8):
        b, r = divmod(c, 4)
        if b >= B:
            continue
        out[b, r * CH:(r + 1) * CH, :] = results[c]["out"].T
    return out


# cached compiled program + jitted runner -----------------------------------

_CACHE = {}


def _get_runner(cfg_key, cfg):
    if cfg_key in _CACHE:
        return _CACHE[cfg_key]
    runner = _runner_from_nc(build(cfg))
    _CACHE[cfg_key] = runner
    return runner


def _runner_from_nc(nc):
    import jax
    from jax.experimental.shard_map import shard_map
    from jax.sharding import Mesh, PartitionSpec
    from concourse import bass2jax

    bass2jax.install_neuronx_cc_hook()

    partition_name = (nc.partition_id_tensor.name
                      if nc.partition_id_tensor else None)
    in_names, out_names, out_avals, zero_shapes = [], [], [], []
    for alloc in nc.m.functions[0].allocations:
        if not isinstance(alloc, mybir.MemoryLocationSet):
            continue
        name = alloc.memorylocations[0].name
        if alloc.kind == "ExternalInput":
            if name != partition_name:
                in_names.append(name)
        elif alloc.kind == "ExternalOutput":
            out_names.append(name)
            shape = tuple(alloc.tensor_shape)
            dtype = mybir.dt.np(alloc.dtype)
            out_avals.append(jax.core.ShapedArray(shape, dtype))
            zero_shapes.append((shape, dtype))
    n_params = len(in_names)
    all_in_names = in_names + out_names
    if partition_name is not None:
        all_in_names = all_in_names + [partition_name]

    def _body(*args):
        operands = list(args)
        if partition_name is not None:
            operands.append(bass2jax.partition_id_tensor())
        outs = bass2jax._bass_exec_p.bind(
            *operands,
            out_avals=tuple(out_avals),
            in_names=tuple(all_in_names),
            out_names=tuple(out_names),
            lowering_input_output_aliases=(),
            sim_require_finite=True,
            sim_require_nnan=True,
            nc=nc,
        )
        return tuple(outs)

    n_outs = len(out_names)
    donate = tuple(range(n_params, n_params + n_outs))
    devices = jax.devices()[:8]
    mesh = Mesh(np.asarray(devices), ("core",))
    in_specs = (PartitionSpec("core"),) * (n_params + n_outs)
    out_specs = (PartitionSpec("core"),) * n_outs
    sharded = jax.jit(
        shard_map(_body, mesh=mesh, in_specs=in_specs, out_specs=out_specs,
                  check_rep=False),
        donate_argnums=donate, keep_unused=True)

    class Runner:
        pass

    runner = Runner()
    runner.sharded = sharded
    runner.nc = nc
    runner.mesh = mesh
    runner.in_names = in_names
    runner.out_names = out_names
    runner.out_avals = out_avals
    runner.zero_shapes = zero_shapes

    def concat_inputs(in_maps):
        return [np.concatenate([np.asarray(m[name]) for m in in_maps],
                               axis=0) for name in in_names]

    def make_zeros():
        return [np.zeros((8 * s[0], *s[1:]), d) for s, d in zero_shapes]

    def split_outputs(out_arrs):
        return [
            {name: np.asarray(out_arrs[i]).reshape(8, *out_avals[i].shape)[c]
             for i, name in enumerate(out_names)}
            for c in range(8)
        ]

    runner.concat_inputs = concat_inputs
    runner.make_zeros = make_zeros
    runner.split_outputs = split_outputs

    def run(in_maps):
        out_arrs = sharded(*concat_inputs(in_maps), *make_zeros())
        return split_outputs(out_arrs)

    runner.run = run

    def make_fast(arg_arrays):
        """AOT-compile the no-donate fast-dispatch variant for the given
        device-resident args (inputs followed by output buffers)."""
        specs = [jax.ShapeDtypeStruct(a.shape, a.dtype, sharding=a.sharding)
                 for a in arg_arrays]
        return bass2jax.fast_dispatch_compile(
            lambda: jax.jit(
                shard_map(_body, mesh=mesh, in_specs=in_specs,
                          out_specs=out_specs, check_rep=False),
                keep_unused=True).lower(*specs).compile())

    runner.make_fast = make_fast
    return runner


def run_cfg(cfg, inputs):
    cfg_key = tuple(sorted(cfg.items()))
    runner = _get_runner(cfg_key, cfg)
    in_maps = make_in_maps(cfg, **inputs)
    results = runner.run(in_maps)
    return assemble(cfg, results, np.asarray(inputs["x"]).shape[0])


def kernel(**inputs):
    return run_cfg(FULL_CFG, inputs)
